# revision 6
# baseline (speedup 1.0000x reference)
"""Trainium2 Bass kernel: single-head causal attention (nn_Head).

Reference computation (per batch b):
    q = x @ Wq.T; k = x @ Wk.T; v = x @ Wv.T          # [T, H]
    S = q @ k.T * D**-0.5, causal-masked               # [T, T]
    P = softmax(S, axis=-1)
    out = P @ v                                        # [T, H]

Shapes: B=16, T=1024, D=768, H=64. f32 in / f32 out.

Sharding: pure data-parallel over batch. 8 cores x 2 batches each; weights
replicated; no collectives. Host shards x, gathers out.

Per-core kernel design (instruction-count-minimized rewrite):
  - x is cast-loaded f32->bf16 by SWDGE (gpsimd queue), then transposed to
    x^T [d, t] by the DMA-transpose XBAR (dma_start_transpose, 16x128 tiles)
    instead of 96 PE transpose matmuls + 96 scalar-engine PSUM copies.
  - Weights are likewise cast-loaded and DMA-transposed; Wq/Wk fuse into one
    [d, 128] stationary so q^T and k^T come from one accumulation group.
  - S^T [s, t] computed per (s-tile j, 512-col chunk c), trimmed to the
    causal staircase; exp on ScalarE writes P^T (bf16) directly; diagonal
    blocks masked by a post-exp multiply with an upper-triangular 0/1 mask.
  - P@V flipped: stationary [v | 1 | 0-pad] [s, 80], moving P^T in 512-wide
    chunks -> out^T [80, t] in PSUM, row 64 = softmax denominators (free).
    Result cast to bf16, DMA-transposed back to natural [t, 80], divided by
    the denominator column on DVE, stored f32.
  - Matmuls bf16 (1 PE cycle/row), accumulation f32 in PSUM. Max-subtraction
    skipped: logits ~N(0, 0.09^2), exp can't overflow; softmax shift-invariant.
"""

import os
import sys

for _p in ("/opt/trn_rl_repo", "/root/.axon_site/_ro/trn_rl_repo"):
    if os.path.isdir(_p) and _p not in sys.path:
        sys.path.insert(0, _p)

import numpy as np

import concourse.bass as bass
import concourse.bacc as bacc
import concourse.mybir as mybir
import concourse.tile as tile
from contextlib import ExitStack
from concourse.masks import make_upper_triangular

B, T, D, H = 16, 1024, 768, 64
NCORES = 8
BL = B // NCORES          # batches per core
TT = T // 128             # 8 t-tiles
KD = D // 128             # 6 d-slices
F32 = mybir.dt.float32
CDT = mybir.dt.bfloat16   # matmul compute dtype
SCALE = float(D) ** -0.5
VP = 80                   # v stationary width: 64 v | 1 ones | 15 zero pad


def build_nc(cdt=CDT):
    nc = bacc.Bacc()
    x = nc.declare_dram_parameter("x", [BL, T, D], F32, isOutput=False)[:]
    wq = nc.declare_dram_parameter("Wq", [H, D], F32, isOutput=False)[:]
    wk = nc.declare_dram_parameter("Wk", [H, D], F32, isOutput=False)[:]
    wv = nc.declare_dram_parameter("Wv", [H, D], F32, isOutput=False)[:]
    out = nc.declare_dram_parameter("out", [BL, T, H], F32, isOutput=True)[:]

    with tile.TileContext(nc) as tc, ExitStack() as ctx:
        const = ctx.enter_context(tc.tile_pool(name="const", bufs=1))
        wpool = ctx.enter_context(tc.tile_pool(name="wpool", bufs=1))
        xnp = ctx.enter_context(tc.tile_pool(name="xnp", bufs=2))
        xtp = ctx.enter_context(tc.tile_pool(name="xtp", bufs=2))
        qkp = ctx.enter_context(tc.tile_pool(name="qkp", bufs=2))
        vsp = ctx.enter_context(tc.tile_pool(name="vsp", bufs=2))
        ptp = ctx.enter_context(tc.tile_pool(name="ptp", bufs=2))
        otp = ctx.enter_context(tc.tile_pool(name="otp", bufs=2))
        onp = ctx.enter_context(tc.tile_pool(name="onp", bufs=2))
        rp = ctx.enter_context(tc.tile_pool(name="rp", bufs=2))
        fop = ctx.enter_context(tc.tile_pool(name="fop", bufs=2))
        ps_qk = ctx.enter_context(tc.tile_pool(name="ps_qk", bufs=2, space="PSUM"))
        ps_v = ctx.enter_context(tc.tile_pool(name="ps_v", bufs=2, space="PSUM"))
        ps_s = ctx.enter_context(tc.tile_pool(name="ps_s", bufs=2, space="PSUM"))
        ps_pv = ctx.enter_context(tc.tile_pool(name="ps_pv", bufs=1, space="PSUM"))

        # 0/1 mask for the diagonal [s, t] block of S^T: 1 where s <= t.
        triu1 = const.tile([128, 128], cdt)
        make_upper_triangular(nc, triu1, val=1.0, diag=True)

        # ---- weights: SWDGE cast load [H, D], DMA-transpose to [d, *] ----
        wqk = wpool.tile([128, KD, 128], cdt)   # [d%128, k, (q h | k h)]
        wvt = wpool.tile([128, KD, H], cdt)     # [d%128, k, h]
        for name, ap, dst in (
            ("q", wq, wqk[:, :, 0:H]),
            ("k", wk, wqk[:, :, H:128]),
            ("v", wv, wvt[:, :, :]),
        ):
            w_s = wpool.tile([H, D], cdt, name=f"w_{name}")
            nc.gpsimd.dma_start(out=w_s, in_=ap)  # SWDGE casts f32->cdt
            nc.sync.dma_start_transpose(dst, w_s[:, :])

        # ---- x: cast-load halves, DMA-transpose to xT [d, (i, k), t] ----
        xTs = []
        for b in range(BL):
            xn = xnp.tile([128, TT, D], cdt, name=f"xn{b}", tag="xn")
            xT = xtp.tile([128, TT, KD, 128], cdt, name=f"xT{b}", tag="xT")
            xv = x[b].rearrange("(i p) d -> p i d", p=128)
            for h in range(2):
                nc.gpsimd.dma_start(
                    out=xn[:, 4 * h:4 * h + 4, :], in_=xv[:, 4 * h:4 * h + 4, :]
                )
                nc.sync.dma_start_transpose(
                    xT[:, 4 * h:4 * h + 4, :, :], xn[:, 4 * h:4 * h + 4, :]
                )
            xTs.append(xT)

        for b in range(BL):
            xT = xTs[b]
            # ---- q^T/k^T [H, T] and v [t, h] per half ----
            qT = qkp.tile([H, T], cdt, name=f"qT{b}", tag="qT")
            kT = qkp.tile([H, T], cdt, name=f"kT{b}", tag="kT")
            vs = vsp.tile([128, TT, VP], cdt, name=f"vs{b}", tag="vs")
            nc.gpsimd.memset(vs[:, :, H:H + 1], 1.0)
            nc.gpsimd.memset(vs[:, :, H + 1:VP], 0.0)
            for c in range(2):
                pqk = ps_qk.tile([128, 512], F32, name="pqk", tag="ps_qk")
                for k in range(KD):
                    nc.tensor.matmul(
                        pqk,
                        wqk[:, k, :],
                        xT[:, 4 * c:4 * c + 4, k, :],
                        start=(k == 0),
                        stop=(k == KD - 1),
                    )
                nc.vector.tensor_copy(qT[:, 512 * c:512 * (c + 1)], pqk[0:H, :])
                nc.vector.tensor_copy(kT[:, 512 * c:512 * (c + 1)], pqk[H:128, :])
                pv = ps_v.tile([128, 4, H], F32, name="pv", tag="ps_v")
                for il in range(4):
                    i = 4 * c + il
                    for k in range(KD):
                        nc.tensor.matmul(
                            pv[:, il, :],
                            xT[:, i, k, :],
                            wvt[:, k, :],
                            start=(k == 0),
                            stop=(k == KD - 1),
                        )
                nc.vector.tensor_copy(vs[:, 4 * c:4 * c + 4, 0:H], pv)

            # ---- S^T staircase chunks + exp -> P^T (bf16), mask diag ----
            pt = ptp.tile([128, TT, T], cdt, name=f"pt{b}", tag="pt")
            for j in range(TT):
                for c in range(j // 4, 2):
                    t0 = max(512 * c, 128 * j)
                    w = 512 * (c + 1) - t0
                    pss = ps_s.tile([128, 512], F32, name="pss", tag="ps_s")
                    nc.tensor.matmul(
                        pss[:, 0:w],
                        kT[:, 128 * j:128 * (j + 1)],
                        qT[:, t0:t0 + w],
                        start=True,
                        stop=True,
                    )
                    nc.scalar.activation(
                        pt[:, j, t0:t0 + w],
                        pss[:, 0:w],
                        mybir.ActivationFunctionType.Exp,
                        scale=SCALE,
                    )
                nc.vector.tensor_tensor(
                    out=pt[:, j, 128 * j:128 * (j + 1)],
                    in0=pt[:, j, 128 * j:128 * (j + 1)],
                    in1=triu1,
                    op=mybir.AluOpType.mult,
                )

            # ---- out^T = [v|1|0]^T @ P^T : [80, T], row 64 = denominators ----
            pav = ps_pv.tile([VP, T], F32, name="pav", tag="ps_pv")
            for c in range(2):
                jmax = 4 * c + 3
                for j in range(jmax + 1):
                    t0 = max(512 * c, 128 * j)
                    nc.tensor.matmul(
                        pav[:, t0:512 * (c + 1)],
                        vs[:, j, :],
                        pt[:, j, t0:512 * (c + 1)],
                        start=(j == 0),
                        stop=(j == jmax),
                    )

            # ---- cast, DMA-transpose to natural, divide by denom, store ----
            otT = otp.tile([VP, T], cdt, name=f"otT{b}", tag="otT")
            for c in range(2):
                nc.vector.tensor_copy(
                    otT[:, 512 * c:512 * (c + 1)], pav[:, 512 * c:512 * (c + 1)]
                )
            on_ = onp.tile([128, TT, VP], cdt, name=f"on{b}", tag="on")
            nc.sync.dma_start_transpose(on_, otT[:, :])
            r = rp.tile([128, TT], F32, name=f"r{b}", tag="r")
            nc.vector.reciprocal(r, on_[:, :, H])
            ot = fop.tile([128, TT, H], F32, name=f"ot{b}", tag="ot")
            for i in range(TT):
                nc.vector.tensor_scalar_mul(
                    ot[:, i, :], on_[:, i, 0:H], r[:, i:i + 1]
                )
            ov = out[b].rearrange("(i p) h -> p i h", p=128)
            nc.sync.dma_start(out=ov, in_=ot)

    nc.finalize()
    return nc


_NC_CACHE = {}


def _get_nc(cdt=CDT):
    key = str(cdt)
    if key not in _NC_CACHE:
        _NC_CACHE[key] = build_nc(cdt)
    return _NC_CACHE[key]


def _make_in_maps(inputs):
    x = np.ascontiguousarray(np.asarray(inputs["x"], dtype=np.float32))
    wq = np.ascontiguousarray(np.asarray(inputs["Wq"], dtype=np.float32))
    wk = np.ascontiguousarray(np.asarray(inputs["Wk"], dtype=np.float32))
    wv = np.ascontiguousarray(np.asarray(inputs["Wv"], dtype=np.float32))
    in_maps = []
    for c in range(NCORES):
        in_maps.append(
            {
                "x": np.ascontiguousarray(x[c * BL:(c + 1) * BL]),
                "Wq": wq,
                "Wk": wk,
                "Wv": wv,
            }
        )
    return in_maps


def kernel(**inputs):
    from concourse.bass_utils import run_bass_kernel_spmd

    nc = _get_nc()
    res = run_bass_kernel_spmd(nc, _make_in_maps(inputs), list(range(NCORES)))
    return np.concatenate([r["out"] for r in res.results], axis=0)


if __name__ == "__main__":
    nc = build_nc()
    print("built OK")


# revision 7
# speedup vs baseline: 1.0930x; 1.0930x over previous
"""Trainium2 Bass kernel: single-head causal attention (nn_Head).

Reference computation (per batch b):
    q = x @ Wq.T; k = x @ Wk.T; v = x @ Wv.T          # [T, H]
    S = q @ k.T * D**-0.5, causal-masked               # [T, T]
    P = softmax(S, axis=-1)
    out = P @ v                                        # [T, H]

Shapes: B=16, T=1024, D=768, H=64. f32 in / f32 out.

Sharding: pure data-parallel over batch. 8 cores x 2 batches each; weights
replicated; no collectives. Host shards x, gathers out.

Per-core kernel design:
  - x^T is produced by the DMA-transpose XBAR (16x128 tiles, bf16) instead of
    PE transposes + scalar copies. x arrives in bf16 two ways, splitting DMA
    load across queues: half via SWDGE cast-load (gpsimd queue), half via
    HWDGE f32 load (sync queue) + on-engine cast (scalar/gpsimd).
  - Every DMA reads/writes its own tile (the tile scheduler treats DMA
    accesses tile-granularly; shared tiles serialize the queues).
  - Wq/Wk fuse into one [d, 128] stationary so q^T/k^T come from one
    accumulation group. Weights go through HWDGE + gpsimd cast + XBAR.
  - S^T [s, t] computed per (s-tile j, 512-col chunk c) trimmed to the causal
    staircase; exp on ScalarE writes P^T (bf16) directly; diagonal blocks
    masked post-exp with an upper-triangular 0/1 multiply on DVE.
  - P@V flipped: stationary [v | 1 | 0-pad] [s, 80], moving P^T in 512-wide
    chunks -> out^T [80, t] in PSUM, row 64 = softmax denominators (free).
    Cast bf16, XBAR-transpose to natural [t, 80], divide by denominator
    column on DVE, store f32.
  - Matmuls bf16, accumulation f32 in PSUM. Max-subtraction skipped: logits
    ~N(0, 0.09^2); exp cannot overflow and softmax is shift-invariant.
"""

import os
import sys

for _p in ("/opt/trn_rl_repo", "/root/.axon_site/_ro/trn_rl_repo"):
    if os.path.isdir(_p) and _p not in sys.path:
        sys.path.insert(0, _p)

import numpy as np

import concourse.bass as bass
import concourse.bacc as bacc
import concourse.mybir as mybir
import concourse.tile as tile
from contextlib import ExitStack
from concourse.masks import make_upper_triangular

B, T, D, H = 16, 1024, 768, 64
NCORES = 8
BL = B // NCORES          # batches per core
TT = T // 128             # 8 t-tiles
KD = D // 128             # 6 d-slices
F32 = mybir.dt.float32
CDT = mybir.dt.bfloat16   # matmul compute dtype
SCALE = float(D) ** -0.5
VP = 80                   # v stationary width: 64 v | 1 ones | 15 zero pad


def build_nc(cdt=CDT):
    nc = bacc.Bacc()
    x = nc.declare_dram_parameter("x", [BL, T, D], F32, isOutput=False)[:]
    wq = nc.declare_dram_parameter("Wq", [H, D], F32, isOutput=False)[:]
    wk = nc.declare_dram_parameter("Wk", [H, D], F32, isOutput=False)[:]
    wv = nc.declare_dram_parameter("Wv", [H, D], F32, isOutput=False)[:]
    out = nc.declare_dram_parameter("out", [BL, T, H], F32, isOutput=True)[:]

    with tile.TileContext(nc) as tc, ExitStack() as ctx:
        const = ctx.enter_context(tc.tile_pool(name="const", bufs=1))
        wpool = ctx.enter_context(tc.tile_pool(name="wpool", bufs=1))
        xnp = ctx.enter_context(tc.tile_pool(name="xnp", bufs=4))
        x32p = ctx.enter_context(tc.tile_pool(name="x32p", bufs=2))
        xtp = ctx.enter_context(tc.tile_pool(name="xtp", bufs=4))
        qkp = ctx.enter_context(tc.tile_pool(name="qkp", bufs=2))
        vsp = ctx.enter_context(tc.tile_pool(name="vsp", bufs=2))
        ptp = ctx.enter_context(tc.tile_pool(name="ptp", bufs=2))
        otp = ctx.enter_context(tc.tile_pool(name="otp", bufs=2))
        onp = ctx.enter_context(tc.tile_pool(name="onp", bufs=2))
        rp = ctx.enter_context(tc.tile_pool(name="rp", bufs=2))
        fop = ctx.enter_context(tc.tile_pool(name="fop", bufs=2))
        ps_qk = ctx.enter_context(tc.tile_pool(name="ps_qk", bufs=2, space="PSUM"))
        ps_v = ctx.enter_context(tc.tile_pool(name="ps_v", bufs=2, space="PSUM"))
        ps_s = ctx.enter_context(tc.tile_pool(name="ps_s", bufs=2, space="PSUM"))
        ps_pv = ctx.enter_context(tc.tile_pool(name="ps_pv", bufs=1, space="PSUM"))

        # 0/1 mask for the diagonal [s, t] block of S^T: 1 where s <= t.
        triu1 = const.tile([128, 128], cdt)
        make_upper_triangular(nc, triu1, val=1.0, diag=True)

        # ---- weights: HWDGE f32 load, gpsimd cast, XBAR transpose ----
        wqk = wpool.tile([128, KD, 128], cdt)   # [d%128, k, (q h | k h)]
        wvt = wpool.tile([128, KD, H], cdt)     # [d%128, k, h]
        wparts = []
        for name, ap, dst in (
            ("q", wq, wqk[:, :, 0:H]),
            ("k", wk, wqk[:, :, H:128]),
            ("v", wv, wvt[:, :, :]),
        ):
            w32 = wpool.tile([H, D], F32, name=f"w32_{name}")
            nc.sync.dma_start(out=w32, in_=ap)
            wparts.append((w32, dst, name))
        wbfs = []
        for w32, dst, name in wparts:
            wbf = wpool.tile([H, D], cdt, name=f"wbf_{name}")
            nc.gpsimd.tensor_copy(wbf, w32)
            wbfs.append((wbf, dst))
        for wbf, dst in wbfs:
            nc.scalar.dma_start_transpose(dst, wbf[:, :])

        # ---- x: per batch, half via SWDGE cast / half via HWDGE f32+cast ----
        # xT[b][h] [128, 4, 6, 128]: chunks (i_local, k), cols = t within tile.
        xvs = [x[b].rearrange("(i p) d -> p i d", p=128) for b in range(BL)]
        xn_sw = []   # bf16 direct from SWDGE (halves h=0)
        xn32 = []    # f32 from HWDGE (halves h=1)
        for b in range(BL):
            t_sw = xnp.tile([128, 4, D], cdt, name=f"xnsw{b}", tag="xnsw")
            nc.gpsimd.dma_start(out=t_sw, in_=xvs[b][:, 0:4, :])
            xn_sw.append(t_sw)
            t_32 = x32p.tile([128, 4, D], F32, name=f"xn32{b}", tag="xn32")
            nc.sync.dma_start(out=t_32, in_=xvs[b][:, 4:8, :])
            xn32.append(t_32)

        # casts for the f32 halves: b0 on scalar, b1 on gpsimd
        xn_cast = []
        for b, eng in ((0, "scalar"), (1, "gpsimd")):
            t_c = xnp.tile([128, 4, D], cdt, name=f"xnc{b}", tag="xnsw")
            if eng == "scalar":
                nc.scalar.copy(t_c, xn32[b])
            else:
                nc.gpsimd.tensor_copy(t_c, xn32[b])
            xn_cast.append(t_c)

        # XBAR transposes: b0h0 via scalar queue, rest via sync queue
        xT = [[None, None] for _ in range(BL)]
        for b in range(BL):
            for h in range(2):
                xT[b][h] = xtp.tile(
                    [128, 4, KD, 128], cdt, name=f"xT{b}{h}", tag="xT"
                )
        nc.scalar.dma_start_transpose(xT[0][0][:, :, :, :], xn_sw[0][:, :, :])
        nc.sync.dma_start_transpose(xT[0][1][:, :, :, :], xn_cast[0][:, :, :])
        nc.scalar.dma_start_transpose(xT[1][0][:, :, :, :], xn_sw[1][:, :, :])
        nc.sync.dma_start_transpose(xT[1][1][:, :, :, :], xn_cast[1][:, :, :])

        for b in range(BL):
            # ---- q^T/k^T [H, T] and v [t, h] per half ----
            qT = qkp.tile([H, T], cdt, name=f"qT{b}", tag="qT")
            kT = qkp.tile([H, T], cdt, name=f"kT{b}", tag="kT")
            vs = vsp.tile([128, TT, VP], cdt, name=f"vs{b}", tag="vs")
            nc.gpsimd.memset(vs[:, :, H:H + 1], 1.0)
            nc.gpsimd.memset(vs[:, :, H + 1:VP], 0.0)
            for c in range(2):
                xTh = xT[b][c]
                pqk = ps_qk.tile([128, 512], F32, name="pqk", tag="ps_qk")
                for k in range(KD):
                    nc.tensor.matmul(
                        pqk,
                        wqk[:, k, :],
                        xTh[:, :, k, :],
                        start=(k == 0),
                        stop=(k == KD - 1),
                    )
                nc.vector.tensor_copy(qT[:, 512 * c:512 * (c + 1)], pqk[0:H, :])
                nc.vector.tensor_copy(kT[:, 512 * c:512 * (c + 1)], pqk[H:128, :])
                pv = ps_v.tile([128, 4, H], F32, name="pv", tag="ps_v")
                for il in range(4):
                    for k in range(KD):
                        nc.tensor.matmul(
                            pv[:, il, :],
                            xTh[:, il, k, :],
                            wvt[:, k, :],
                            start=(k == 0),
                            stop=(k == KD - 1),
                        )
                nc.vector.tensor_copy(vs[:, 4 * c:4 * c + 4, 0:H], pv)

            # ---- S^T staircase chunks + exp -> P^T (bf16), mask diag ----
            pt = ptp.tile([128, TT, T], cdt, name=f"pt{b}", tag="pt")
            for j in range(TT):
                for c in range(j // 4, 2):
                    t0 = max(512 * c, 128 * j)
                    w = 512 * (c + 1) - t0
                    pss = ps_s.tile([128, 512], F32, name="pss", tag="ps_s")
                    nc.tensor.matmul(
                        pss[:, 0:w],
                        kT[:, 128 * j:128 * (j + 1)],
                        qT[:, t0:t0 + w],
                        start=True,
                        stop=True,
                    )
                    nc.scalar.activation(
                        pt[:, j, t0:t0 + w],
                        pss[:, 0:w],
                        mybir.ActivationFunctionType.Exp,
                        scale=SCALE,
                    )
                nc.vector.tensor_tensor(
                    out=pt[:, j, 128 * j:128 * (j + 1)],
                    in0=pt[:, j, 128 * j:128 * (j + 1)],
                    in1=triu1,
                    op=mybir.AluOpType.mult,
                )

            # ---- out^T = [v|1|0]^T @ P^T : [80, T], row 64 = denominators ----
            pav = ps_pv.tile([VP, T], F32, name="pav", tag="ps_pv")
            for c in range(2):
                jmax = 4 * c + 3
                for j in range(jmax + 1):
                    t0 = max(512 * c, 128 * j)
                    nc.tensor.matmul(
                        pav[:, t0:512 * (c + 1)],
                        vs[:, j, :],
                        pt[:, j, t0:512 * (c + 1)],
                        start=(j == 0),
                        stop=(j == jmax),
                    )

            # ---- cast, XBAR transpose to natural, divide by denom, store ----
            otT = otp.tile([VP, T], cdt, name=f"otT{b}", tag="otT")
            for c in range(2):
                nc.vector.tensor_copy(
                    otT[:, 512 * c:512 * (c + 1)], pav[:, 512 * c:512 * (c + 1)]
                )
            on_ = onp.tile([128, TT, VP], cdt, name=f"on{b}", tag="on")
            nc.sync.dma_start_transpose(on_, otT[:, :])
            r = rp.tile([128, TT], F32, name=f"r{b}", tag="r")
            nc.vector.reciprocal(r, on_[:, :, H])
            ot = fop.tile([128, TT, H], F32, name=f"ot{b}", tag="ot")
            for i in range(TT):
                nc.vector.tensor_scalar_mul(
                    ot[:, i, :], on_[:, i, 0:H], r[:, i:i + 1]
                )
            ov = out[b].rearrange("(i p) h -> p i h", p=128)
            nc.sync.dma_start(out=ov, in_=ot)

    nc.finalize()
    return nc


_NC_CACHE = {}


def _get_nc(cdt=CDT):
    key = str(cdt)
    if key not in _NC_CACHE:
        _NC_CACHE[key] = build_nc(cdt)
    return _NC_CACHE[key]


def _make_in_maps(inputs):
    x = np.ascontiguousarray(np.asarray(inputs["x"], dtype=np.float32))
    wq = np.ascontiguousarray(np.asarray(inputs["Wq"], dtype=np.float32))
    wk = np.ascontiguousarray(np.asarray(inputs["Wk"], dtype=np.float32))
    wv = np.ascontiguousarray(np.asarray(inputs["Wv"], dtype=np.float32))
    in_maps = []
    for c in range(NCORES):
        in_maps.append(
            {
                "x": np.ascontiguousarray(x[c * BL:(c + 1) * BL]),
                "Wq": wq,
                "Wk": wk,
                "Wv": wv,
            }
        )
    return in_maps


def kernel(**inputs):
    from concourse.bass_utils import run_bass_kernel_spmd

    nc = _get_nc()
    res = run_bass_kernel_spmd(nc, _make_in_maps(inputs), list(range(NCORES)))
    return np.concatenate([r["out"] for r in res.results], axis=0)


if __name__ == "__main__":
    nc = build_nc()
    print("built OK")


# revision 10
# speedup vs baseline: 1.1076x; 1.0134x over previous
"""Trainium2 Bass kernel: single-head causal attention (nn_Head).

Reference computation (per batch b):
    q = x @ Wq.T; k = x @ Wk.T; v = x @ Wv.T          # [T, H]
    S = q @ k.T * D**-0.5, causal-masked               # [T, T]
    P = softmax(S, axis=-1)
    out = P @ v                                        # [T, H]

Shapes: B=16, T=1024, D=768, H=64. f32 in / f32 out.

Sharding: pure data-parallel over batch. 8 cores x 2 batches each; weights
replicated; no collectives. Host shards x, gathers out.

Per-core kernel design:
  - x^T is produced by the DMA-transpose XBAR (16x128 tiles, bf16) instead of
    PE transposes + scalar copies. x arrives in bf16 two ways, splitting DMA
    load across queues: half via SWDGE cast-load (gpsimd queue), half via
    HWDGE f32 load (sync queue) + on-engine cast (scalar/gpsimd).
  - Every DMA reads/writes its own tile (the tile scheduler treats DMA
    accesses tile-granularly; shared tiles serialize the queues).
  - Wq/Wk fuse into one [d, 128] stationary so q^T/k^T come from one
    accumulation group. Weights go through HWDGE + gpsimd cast + XBAR.
  - S^T [s, t] computed per (s-tile j, 512-col chunk c) trimmed to the causal
    staircase; exp on ScalarE writes P^T (bf16) directly; diagonal blocks
    masked post-exp with an upper-triangular 0/1 multiply on DVE.
  - P@V flipped: stationary [v | 1 | 0-pad] [s, 80], moving P^T in 512-wide
    chunks -> out^T [80, t] in PSUM, row 64 = softmax denominators (free).
    Cast bf16, XBAR-transpose to natural [t, 80], divide by denominator
    column on DVE, store f32.
  - Matmuls bf16, accumulation f32 in PSUM. Max-subtraction skipped: logits
    ~N(0, 0.09^2); exp cannot overflow and softmax is shift-invariant.
"""

import os
import sys

for _p in ("/opt/trn_rl_repo", "/root/.axon_site/_ro/trn_rl_repo"):
    if os.path.isdir(_p) and _p not in sys.path:
        sys.path.insert(0, _p)

import numpy as np

import concourse.bass as bass
import concourse.bacc as bacc
import concourse.mybir as mybir
import concourse.tile as tile
from contextlib import ExitStack
from concourse.masks import make_upper_triangular

B, T, D, H = 16, 1024, 768, 64
NCORES = 8
BL = B // NCORES          # batches per core
TT = T // 128             # 8 t-tiles
KD = D // 128             # 6 d-slices
F32 = mybir.dt.float32
CDT = mybir.dt.bfloat16   # matmul compute dtype
SCALE = float(D) ** -0.5
VP = 80                   # v stationary width: 64 v | 1 ones | 15 zero pad


def build_nc(cdt=CDT):
    nc = bacc.Bacc()
    x = nc.declare_dram_parameter("x", [BL, T, D], F32, isOutput=False)[:]
    wq = nc.declare_dram_parameter("Wq", [H, D], F32, isOutput=False)[:]
    wk = nc.declare_dram_parameter("Wk", [H, D], F32, isOutput=False)[:]
    wv = nc.declare_dram_parameter("Wv", [H, D], F32, isOutput=False)[:]
    out = nc.declare_dram_parameter("out", [BL, T, H], F32, isOutput=True)[:]

    with tile.TileContext(nc) as tc, ExitStack() as ctx:
        const = ctx.enter_context(tc.tile_pool(name="const", bufs=1))
        wpool = ctx.enter_context(tc.tile_pool(name="wpool", bufs=1))
        xnp = ctx.enter_context(tc.tile_pool(name="xnp", bufs=4))
        x32p = ctx.enter_context(tc.tile_pool(name="x32p", bufs=4))
        xtp = ctx.enter_context(tc.tile_pool(name="xtp", bufs=4))
        qkp = ctx.enter_context(tc.tile_pool(name="qkp", bufs=2))
        vsp = ctx.enter_context(tc.tile_pool(name="vsp", bufs=2))
        ptp = ctx.enter_context(tc.tile_pool(name="ptp", bufs=2))
        otp = ctx.enter_context(tc.tile_pool(name="otp", bufs=2))
        onp = ctx.enter_context(tc.tile_pool(name="onp", bufs=2))
        rp = ctx.enter_context(tc.tile_pool(name="rp", bufs=2))
        fop = ctx.enter_context(tc.tile_pool(name="fop", bufs=2))
        ps_qk = ctx.enter_context(tc.tile_pool(name="ps_qk", bufs=2, space="PSUM"))
        ps_v = ctx.enter_context(tc.tile_pool(name="ps_v", bufs=2, space="PSUM"))
        ps_s = ctx.enter_context(tc.tile_pool(name="ps_s", bufs=2, space="PSUM"))
        ps_pv = ctx.enter_context(tc.tile_pool(name="ps_pv", bufs=1, space="PSUM"))

        # 0/1 mask for the diagonal [s, t] block of S^T: 1 where s <= t.
        triu1 = const.tile([128, 128], cdt)
        make_upper_triangular(nc, triu1, val=1.0, diag=True)

        # ---- weights: sync f32 load, gpsimd cast, XBAR transpose (qAct) ----
        wqk = wpool.tile([128, KD, 128], cdt)   # [d%128, k, (q h | k h)]
        wvt = wpool.tile([128, KD, H], cdt)     # [d%128, k, h]
        wparts = []
        for name, ap, dst in (
            ("q", wq, wqk[:, :, 0:H]),
            ("k", wk, wqk[:, :, H:128]),
            ("v", wv, wvt[:, :, :]),
        ):
            w32 = wpool.tile([H, D], F32, name=f"w32_{name}")
            nc.sync.dma_start(out=w32, in_=ap)
            wparts.append((w32, dst, name))

        # ---- x: all four halves as HWDGE f32 loads, split across queues ----
        # h=0 halves via sync queue, h=1 halves via scalar queue.
        xvs = [x[b].rearrange("(i p) d -> p i d", p=128) for b in range(BL)]
        xn32 = [[None, None] for _ in range(BL)]
        for b in range(BL):
            for h, eng in ((0, nc.sync), (1, nc.scalar)):
                t_32 = x32p.tile([128, 4, D], F32, name=f"xn32_{b}{h}", tag="x32")
                eng.dma_start(out=t_32, in_=xvs[b][:, 4 * h:4 * h + 4, :])
                xn32[b][h] = t_32

        # weight casts on gpsimd, then XBAR transposes from the scalar queue
        wbfs = []
        for w32, dst, name in wparts:
            wbf = wpool.tile([H, D], cdt, name=f"wbf_{name}")
            nc.gpsimd.tensor_copy(wbf, w32)
            wbfs.append((wbf, dst))
        for wbf, dst in wbfs:
            nc.scalar.dma_start_transpose(dst, wbf[:, :])

        # x casts: spread across ACT / DVE / ACT / Pool
        cast_eng = {(0, 0): "act", (0, 1): "dve", (1, 0): "pool", (1, 1): "act"}
        xn_bf = [[None, None] for _ in range(BL)]
        for (b, h), eng in cast_eng.items():
            t_c = xnp.tile([128, 4, D], cdt, name=f"xnbf_{b}{h}", tag="xnbf")
            xn_bf[b][h] = t_c
        nc.scalar.copy(xn_bf[0][0], xn32[0][0])
        nc.vector.tensor_copy(xn_bf[0][1], xn32[0][1])
        nc.gpsimd.tensor_copy(xn_bf[1][0], xn32[1][0])

        # XBAR transposes; xT[b][h] [128, 4, 6, 128]: chunks (i_local, k)
        xT = [[None, None] for _ in range(BL)]
        for b in range(BL):
            for h in range(2):
                xT[b][h] = xtp.tile(
                    [128, 4, KD, 128], cdt, name=f"xT{b}{h}", tag="xT"
                )
        nc.sync.dma_start_transpose(xT[0][0][:, :, :, :], xn_bf[0][0][:, :, :])
        nc.scalar.dma_start_transpose(xT[0][1][:, :, :, :], xn_bf[0][1][:, :, :])
        nc.scalar.copy(xn_bf[1][1], xn32[1][1])
        nc.sync.dma_start_transpose(xT[1][0][:, :, :, :], xn_bf[1][0][:, :, :])
        nc.scalar.dma_start_transpose(xT[1][1][:, :, :, :], xn_bf[1][1][:, :, :])

        for b in range(BL):
            # ---- q^T/k^T [H, T] and v [t, h] per half ----
            qT = qkp.tile([H, T], cdt, name=f"qT{b}", tag="qT")
            kT = qkp.tile([H, T], cdt, name=f"kT{b}", tag="kT")
            vs = vsp.tile([128, TT, VP], cdt, name=f"vs{b}", tag="vs")
            nc.gpsimd.memset(vs[:, :, H:H + 1], 1.0)
            nc.gpsimd.memset(vs[:, :, H + 1:VP], 0.0)
            for c in range(2):
                xTh = xT[b][c]
                pqk = ps_qk.tile([128, 512], F32, name="pqk", tag="ps_qk")
                for k in range(KD):
                    nc.tensor.matmul(
                        pqk,
                        wqk[:, k, :],
                        xTh[:, :, k, :],
                        start=(k == 0),
                        stop=(k == KD - 1),
                    )
                nc.vector.tensor_copy(qT[:, 512 * c:512 * (c + 1)], pqk[0:H, :])
                nc.vector.tensor_copy(kT[:, 512 * c:512 * (c + 1)], pqk[H:128, :])
                pv = ps_v.tile([128, 4, H], F32, name="pv", tag="ps_v")
                for il in range(4):
                    for k in range(KD):
                        nc.tensor.matmul(
                            pv[:, il, :],
                            xTh[:, il, k, :],
                            wvt[:, k, :],
                            start=(k == 0),
                            stop=(k == KD - 1),
                        )
                nc.vector.tensor_copy(vs[:, 4 * c:4 * c + 4, 0:H], pv)

            # ---- S^T staircase chunks + exp -> P^T (bf16), mask diag ----
            pt = ptp.tile([128, TT, T], cdt, name=f"pt{b}", tag="pt")
            for j in range(TT):
                for c in range(j // 4, 2):
                    t0 = max(512 * c, 128 * j)
                    w = 512 * (c + 1) - t0
                    pss = ps_s.tile([128, 512], F32, name="pss", tag="ps_s")
                    nc.tensor.matmul(
                        pss[:, 0:w],
                        kT[:, 128 * j:128 * (j + 1)],
                        qT[:, t0:t0 + w],
                        start=True,
                        stop=True,
                    )
                    nc.scalar.activation(
                        pt[:, j, t0:t0 + w],
                        pss[:, 0:w],
                        mybir.ActivationFunctionType.Exp,
                        scale=SCALE,
                    )
                nc.vector.tensor_tensor(
                    out=pt[:, j, 128 * j:128 * (j + 1)],
                    in0=pt[:, j, 128 * j:128 * (j + 1)],
                    in1=triu1,
                    op=mybir.AluOpType.mult,
                )

            # ---- out^T = [v|1|0]^T @ P^T : [80, T], row 64 = denominators ----
            pav = ps_pv.tile([VP, T], F32, name="pav", tag="ps_pv")
            for c in range(2):
                jmax = 4 * c + 3
                for j in range(jmax + 1):
                    t0 = max(512 * c, 128 * j)
                    nc.tensor.matmul(
                        pav[:, t0:512 * (c + 1)],
                        vs[:, j, :],
                        pt[:, j, t0:512 * (c + 1)],
                        start=(j == 0),
                        stop=(j == jmax),
                    )

            # ---- cast, XBAR transpose to natural, divide by denom, store ----
            otT = otp.tile([VP, T], cdt, name=f"otT{b}", tag="otT")
            for c in range(2):
                nc.vector.tensor_copy(
                    otT[:, 512 * c:512 * (c + 1)], pav[:, 512 * c:512 * (c + 1)]
                )
            on_ = onp.tile([128, TT, VP], cdt, name=f"on{b}", tag="on")
            pv_eng = nc.sync if b == 0 else nc.scalar
            pv_eng.dma_start_transpose(on_, otT[:, :])
            r = rp.tile([128, TT], F32, name=f"r{b}", tag="r")
            nc.vector.reciprocal(r, on_[:, :, H])
            ot = fop.tile([128, TT, H], F32, name=f"ot{b}", tag="ot")
            for i in range(TT):
                nc.vector.tensor_scalar_mul(
                    ot[:, i, :], on_[:, i, 0:H], r[:, i:i + 1]
                )
            ov = out[b].rearrange("(i p) h -> p i h", p=128)
            nc.sync.dma_start(out=ov, in_=ot)

    nc.finalize()
    return nc


_NC_CACHE = {}


def _get_nc(cdt=CDT):
    key = str(cdt)
    if key not in _NC_CACHE:
        _NC_CACHE[key] = build_nc(cdt)
    return _NC_CACHE[key]


def _make_in_maps(inputs):
    x = np.ascontiguousarray(np.asarray(inputs["x"], dtype=np.float32))
    wq = np.ascontiguousarray(np.asarray(inputs["Wq"], dtype=np.float32))
    wk = np.ascontiguousarray(np.asarray(inputs["Wk"], dtype=np.float32))
    wv = np.ascontiguousarray(np.asarray(inputs["Wv"], dtype=np.float32))
    in_maps = []
    for c in range(NCORES):
        in_maps.append(
            {
                "x": np.ascontiguousarray(x[c * BL:(c + 1) * BL]),
                "Wq": wq,
                "Wk": wk,
                "Wv": wv,
            }
        )
    return in_maps


def kernel(**inputs):
    from concourse.bass_utils import run_bass_kernel_spmd

    nc = _get_nc()
    res = run_bass_kernel_spmd(nc, _make_in_maps(inputs), list(range(NCORES)))
    return np.concatenate([r["out"] for r in res.results], axis=0)


if __name__ == "__main__":
    nc = build_nc()
    print("built OK")


# revision 19
# speedup vs baseline: 1.4354x; 1.2960x over previous
"""Trainium2 Bass kernel: single-head causal attention (nn_Head).

Reference computation (per batch b):
    q = x @ Wq.T; k = x @ Wk.T; v = x @ Wv.T          # [T, H]
    S = q @ k.T * D**-0.5, causal-masked               # [T, T]
    P = softmax(S, axis=-1)
    out = P @ v                                        # [T, H]

Shapes: B=16, T=1024, D=768, H=64. f32 in / f32 out.

Sharding: pure data-parallel over batch. 8 cores x 2 batches each; weights
replicated; no collectives. Host shards x, gathers out.

Per-core kernel design:
  - x^T is produced by the DMA-transpose XBAR (16x128 tiles, bf16) instead of
    PE transposes + scalar copies. x arrives in bf16 two ways, splitting DMA
    load across queues: half via SWDGE cast-load (gpsimd queue), half via
    HWDGE f32 load (sync queue) + on-engine cast (scalar/gpsimd).
  - Every DMA reads/writes its own tile (the tile scheduler treats DMA
    accesses tile-granularly; shared tiles serialize the queues).
  - Wq/Wk fuse into one [d, 128] stationary so q^T/k^T come from one
    accumulation group. Weights go through HWDGE + gpsimd cast + XBAR.
  - S^T [s, t] computed per (s-tile j, 512-col chunk c) trimmed to the causal
    staircase; exp on ScalarE writes P^T (bf16) directly; diagonal blocks
    masked post-exp with an upper-triangular 0/1 multiply on DVE.
  - P@V flipped: stationary [v | 1 | 0-pad] [s, 80], moving P^T in 512-wide
    chunks -> out^T [80, t] in PSUM, row 64 = softmax denominators (free).
    Cast bf16, XBAR-transpose to natural [t, 80], divide by denominator
    column on DVE, store f32.
  - Matmuls bf16, accumulation f32 in PSUM. Max-subtraction skipped: logits
    ~N(0, 0.09^2); exp cannot overflow and softmax is shift-invariant.
"""

import os
import sys

for _p in ("/opt/trn_rl_repo", "/root/.axon_site/_ro/trn_rl_repo"):
    if os.path.isdir(_p) and _p not in sys.path:
        sys.path.insert(0, _p)

import numpy as np

import concourse.bass as bass
import concourse.bacc as bacc
import concourse.mybir as mybir
import concourse.tile as tile
from contextlib import ExitStack
from concourse.masks import make_identity, make_upper_triangular

B, T, D, H = 16, 1024, 768, 64
NCORES = 8
BL = B // NCORES          # batches per core
TT = T // 128             # 8 t-tiles
KD = D // 128             # 6 d-slices
F32 = mybir.dt.float32
CDT = mybir.dt.bfloat16   # matmul compute dtype
SCALE = float(D) ** -0.5
VP = 80                   # v stationary width: 64 v | 1 ones | 15 zero pad


def build_nc(cdt=CDT):
    nc = bacc.Bacc()
    x = nc.declare_dram_parameter("x", [BL, T, D], F32, isOutput=False)[:]
    wq = nc.declare_dram_parameter("Wq", [H, D], F32, isOutput=False)[:]
    wk = nc.declare_dram_parameter("Wk", [H, D], F32, isOutput=False)[:]
    wv = nc.declare_dram_parameter("Wv", [H, D], F32, isOutput=False)[:]
    out = nc.declare_dram_parameter("out", [BL, T, H], F32, isOutput=True)[:]

    with tile.TileContext(nc) as tc, ExitStack() as ctx:
        const = ctx.enter_context(tc.tile_pool(name="const", bufs=1))
        wpool = ctx.enter_context(tc.tile_pool(name="wpool", bufs=1))
        xnp = ctx.enter_context(tc.tile_pool(name="xnp", bufs=4))
        x32p = ctx.enter_context(tc.tile_pool(name="x32p", bufs=4))
        xtp = ctx.enter_context(tc.tile_pool(name="xtp", bufs=4))
        qkp = ctx.enter_context(tc.tile_pool(name="qkp", bufs=2))
        vsp = ctx.enter_context(tc.tile_pool(name="vsp", bufs=2))
        ptp = ctx.enter_context(tc.tile_pool(name="ptp", bufs=2))
        otp = ctx.enter_context(tc.tile_pool(name="otp", bufs=2))
        onp = ctx.enter_context(tc.tile_pool(name="onp", bufs=2))
        rp = ctx.enter_context(tc.tile_pool(name="rp", bufs=2))
        fop = ctx.enter_context(tc.tile_pool(name="fop", bufs=2))
        ps_qk = ctx.enter_context(tc.tile_pool(name="ps_qk", bufs=2, space="PSUM"))
        ps_v = ctx.enter_context(tc.tile_pool(name="ps_v", bufs=2, space="PSUM"))
        ps_s = ctx.enter_context(tc.tile_pool(name="ps_s", bufs=2, space="PSUM"))
        ps_pv = ctx.enter_context(tc.tile_pool(name="ps_pv", bufs=1, space="PSUM"))

        # 0/1 mask for the diagonal [s, t] block of S^T: 1 where s <= t.
        triu1 = const.tile([128, 128], cdt)
        make_upper_triangular(nc, triu1, val=1.0, diag=True)
        ident = const.tile([64, 64], cdt)
        make_identity(nc, ident)

        # ---- weights: SWDGE cast-load (gpsimd queue is otherwise idle) ----
        wqk = wpool.tile([128, KD, 128], cdt)   # [d%128, k, (q h | k h)]
        wvt = wpool.tile([128, KD, H], cdt)     # [d%128, k, h]
        wparts = []
        for name, ap, dst in (
            ("q", wq, wqk[:, :, 0:H]),
            ("k", wk, wqk[:, :, H:128]),
            ("v", wv, wvt[:, :, :]),
        ):
            wst = wpool.tile([H, D], cdt, name=f"wst_{name}")
            nc.gpsimd.dma_start(out=wst, in_=ap)
            wparts.append((wst, dst, name))

        # ---- x: all four halves as HWDGE f32 loads, split across queues ----
        # h=0 halves via sync queue, h=1 halves via scalar queue.
        xvs = [x[b].rearrange("(i p) d -> p i d", p=128) for b in range(BL)]
        xn32 = [[None, None] for _ in range(BL)]
        for b in range(BL):
            for h, eng in ((0, nc.sync), (1, nc.scalar)):
                t_32 = x32p.tile([128, 4, D], F32, name=f"xn32_{b}{h}", tag="x32")
                eng.dma_start(out=t_32, in_=xvs[b][:, 4 * h:4 * h + 4, :])
                xn32[b][h] = t_32

        # ---- weight transposes on PE (bf16), copy on DVE ----
        for wst, dst, name in wparts:
            pw = ps_s.tile([128, KD, H], cdt, name=f"pw_{name}", tag="ps_s")
            for k in range(KD):
                nc.tensor.transpose(
                    pw[:, k, :],
                    wst[:, 128 * k:128 * (k + 1)],
                    ident,
                )
            nc.vector.tensor_copy(dst, pw)

        # x casts: ACT for b0h0/b1h1, DVE for b0h1/b1h0
        xn_bf = [[None, None] for _ in range(BL)]
        for b in range(BL):
            for h in range(2):
                xn_bf[b][h] = xnp.tile(
                    [128, 4, D], cdt, name=f"xnbf_{b}{h}", tag="xnbf"
                )
        nc.scalar.copy(xn_bf[0][0], xn32[0][0])
        nc.vector.tensor_copy(xn_bf[0][1], xn32[0][1])
        nc.vector.tensor_copy(xn_bf[1][0], xn32[1][0])
        nc.scalar.copy(xn_bf[1][1], xn32[1][1])

        # XBAR transposes; xT[b][h] [128, 4, 6, 128]: chunks (i_local, k)
        # h=0 tiles via sync queue, h=1 via scalar queue, in data-arrival order
        xT = [[None, None] for _ in range(BL)]
        for b in range(BL):
            for h in range(2):
                xT[b][h] = xtp.tile(
                    [128, 4, KD, 128], cdt, name=f"xT{b}{h}", tag="xT"
                )
        nc.sync.dma_start_transpose(xT[0][0][:, :, :, :], xn_bf[0][0][:, :, :])
        nc.sync.dma_start_transpose(xT[0][1][:, :, :, :], xn_bf[0][1][:, :, :])
        nc.sync.dma_start_transpose(xT[1][0][:, :, :, :], xn_bf[1][0][:, :, :])
        nc.sync.dma_start_transpose(xT[1][1][:, :, :, :], xn_bf[1][1][:, :, :])

        for b in range(BL):
            # ---- q^T/k^T [H, T] and v [t, h] per half ----
            qT = qkp.tile([H, T], cdt, name=f"qT{b}", tag="qT")
            kT = qkp.tile([H, T], cdt, name=f"kT{b}", tag="kT")
            vs = vsp.tile([128, TT, VP], cdt, name=f"vs{b}", tag="vs")
            nc.gpsimd.memset(vs[:, :, H:H + 1], 1.0)
            nc.gpsimd.memset(vs[:, :, H + 1:VP], 0.0)
            for c in range(2):
                xTh = xT[b][c]
                pqk = ps_qk.tile([128, 512], F32, name="pqk", tag="ps_qk")
                for k in range(KD):
                    nc.tensor.matmul(
                        pqk,
                        wqk[:, k, :],
                        xTh[:, :, k, :],
                        start=(k == 0),
                        stop=(k == KD - 1),
                    )
                nc.vector.tensor_copy(qT[:, 512 * c:512 * (c + 1)], pqk[0:H, :])
                nc.vector.tensor_copy(kT[:, 512 * c:512 * (c + 1)], pqk[H:128, :])
                pv = ps_v.tile([128, 4, H], F32, name="pv", tag="ps_v")
                for il in range(4):
                    for k in range(KD):
                        nc.tensor.matmul(
                            pv[:, il, :],
                            xTh[:, il, k, :],
                            wvt[:, k, :],
                            start=(k == 0),
                            stop=(k == KD - 1),
                        )
                nc.vector.tensor_copy(vs[:, 4 * c:4 * c + 4, 0:H], pv)

            # ---- S^T staircase chunks + exp -> P^T (bf16), mask diag ----
            pt = ptp.tile([128, TT, T], cdt, name=f"pt{b}", tag="pt")
            for j in range(TT):
                for c in range(j // 4, 2):
                    t0 = max(512 * c, 128 * j)
                    w = 512 * (c + 1) - t0
                    pss = ps_s.tile([128, 512], F32, name="pss", tag="ps_s")
                    nc.tensor.matmul(
                        pss[:, 0:w],
                        kT[:, 128 * j:128 * (j + 1)],
                        qT[:, t0:t0 + w],
                        start=True,
                        stop=True,
                    )
                    nc.scalar.activation(
                        pt[:, j, t0:t0 + w],
                        pss[:, 0:w],
                        mybir.ActivationFunctionType.Exp,
                        scale=SCALE,
                    )
                nc.vector.tensor_tensor(
                    out=pt[:, j, 128 * j:128 * (j + 1)],
                    in0=pt[:, j, 128 * j:128 * (j + 1)],
                    in1=triu1,
                    op=mybir.AluOpType.mult,
                )

            # ---- out^T = [v|1|0]^T @ P^T : [80, T], row 64 = denominators ----
            pav = ps_pv.tile([VP, T], F32, name="pav", tag="ps_pv")
            for c in range(2):
                jmax = 4 * c + 3
                for j in range(jmax + 1):
                    t0 = max(512 * c, 128 * j)
                    nc.tensor.matmul(
                        pav[:, t0:512 * (c + 1)],
                        vs[:, j, :],
                        pt[:, j, t0:512 * (c + 1)],
                        start=(j == 0),
                        stop=(j == jmax),
                    )

            # ---- cast, XBAR transpose to natural, divide by denom, store ----
            otT = otp.tile([VP, T], cdt, name=f"otT{b}", tag="otT")
            for c in range(2):
                nc.vector.tensor_copy(
                    otT[:, 512 * c:512 * (c + 1)], pav[:, 512 * c:512 * (c + 1)]
                )
            on_ = onp.tile([128, TT, VP], cdt, name=f"on{b}", tag="on")
            pv_eng = nc.sync
            pv_eng.dma_start_transpose(on_, otT[:, :])
            r = rp.tile([128, TT], F32, name=f"r{b}", tag="r")
            nc.vector.reciprocal(r, on_[:, :, H])
            ot = fop.tile([128, TT, H], F32, name=f"ot{b}", tag="ot")
            for i in range(TT):
                nc.vector.tensor_scalar_mul(
                    ot[:, i, :], on_[:, i, 0:H], r[:, i:i + 1]
                )
            ov = out[b].rearrange("(i p) h -> p i h", p=128)
            nc.sync.dma_start(out=ov, in_=ot)

    nc.finalize()
    return nc


_NC_CACHE = {}


def _get_nc(cdt=CDT):
    key = str(cdt)
    if key not in _NC_CACHE:
        _NC_CACHE[key] = build_nc(cdt)
    return _NC_CACHE[key]


def _make_in_maps(inputs):
    x = np.ascontiguousarray(np.asarray(inputs["x"], dtype=np.float32))
    wq = np.ascontiguousarray(np.asarray(inputs["Wq"], dtype=np.float32))
    wk = np.ascontiguousarray(np.asarray(inputs["Wk"], dtype=np.float32))
    wv = np.ascontiguousarray(np.asarray(inputs["Wv"], dtype=np.float32))
    in_maps = []
    for c in range(NCORES):
        in_maps.append(
            {
                "x": np.ascontiguousarray(x[c * BL:(c + 1) * BL]),
                "Wq": wq,
                "Wk": wk,
                "Wv": wv,
            }
        )
    return in_maps


def kernel(**inputs):
    from concourse.bass_utils import run_bass_kernel_spmd

    nc = _get_nc()
    res = run_bass_kernel_spmd(nc, _make_in_maps(inputs), list(range(NCORES)))
    return np.concatenate([r["out"] for r in res.results], axis=0)


if __name__ == "__main__":
    nc = build_nc()
    print("built OK")
